# revision 8
# baseline (speedup 1.0000x reference)
"""Trainium2 Bass kernel for a continuous-time diagonal SSM layer (S5/S4D-style).

Math (see reference):
    A = exp(Lambda * step)                 (P,) complex, |A| = r, arg = theta
    Bu[t] = B_bar @ u[t]                   (L, P) complex
    x[t]  = A * x[t-1] + Bu[t]             diagonal complex scan
    ys    = 2 Re(C_tilde @ x) + D * u

Kernel strategy (8 cores, sequence-parallel over L):
  * core i owns a contiguous span of LC = L/8 timesteps, chunked by T=512.
  * rotation trick: with A = r e^{i.th}, define y[t] = e^{-i.th.t} x[t].
    Then y obeys a REAL per-partition recurrence y[t] = r y[t-1] + w[t],
    w[t] = e^{-i.th.t} Bu[t], which maps onto the DVE tensor_tensor_scan
    instruction (state = data0*state + data1 along the free dim).
  * layout: state-major [128 modes (partitions) x time (free)].  u arrives
    in natural (LC, H) layout and is transposed on-chip via PE-transpose
    (identity matmul) into u^T tiles; Bu^T comes from PE matmuls vs u^T;
    the output matmul produces time-major ys directly (lhsT = x tiles),
    D*u is folded in as a diagonal matmul from u^T.  Phase tables
    (cos/sin of th*s, s in [0,T)) are chunk-local, host-precomputed in
    f64; chunk carries chain through the scan `initial` operand after a
    tiny per-chunk basis rotation.
  * cross-core carry: each core AllGathers its span-final scan state (1KB),
    combines them with host-precomputed decay weights, and adds the
    correction r^{s+1}*G into y before the output projection.
  * dtypes: bf16 elementwise domain, f32 scan multiplier (bf16 cannot
    represent slow decay rates like r=0.99995), float32r matmuls.

Host strategy (the per-call wall-clock is dominated by host work, not the
~100us device program):
  * the jax/PJRT dispatch (shard_map over 8 cores -> bass_exec custom
    call) is built ONCE and cached; steady-state calls hit the jit C++
    fast path instead of re-tracing + re-lowering per call.
  * all parameter-derived tables are cached keyed by the raw param bytes
    and kept device-resident (device_put once); only `u` (the activation)
    crosses host->device per call, with no host-side reshape/transpose
    (the global input IS input_sequence).
  * the previous call's device-resident output is donated as the next
    call's output buffer, so no zero-buffer upload per call.
"""

import numpy as np
import ml_dtypes
import jax
from jax.experimental.shard_map import shard_map
from jax.sharding import Mesh, NamedSharding, PartitionSpec

import concourse.bass as bass
import concourse.bacc as bacc
import concourse.tile as tile
import concourse.mybir as mybir
from concourse.bass2jax import (
    _bass_exec_p,
    install_neuronx_cc_hook,
    partition_id_tensor,
)

F32 = mybir.dt.float32
F32R = mybir.dt.float32r
BF16 = mybir.dt.bfloat16
NPBF16 = ml_dtypes.bfloat16
AX = mybir.AxisListType.X
MUL = mybir.AluOpType.mult
ADD = mybir.AluOpType.add

L, H, P = 32768, 256, 128
NCORES = 8
LC = L // NCORES          # 4096 timesteps per core
T = 512                   # chunk length
NCH = LC // T             # 8 chunks per core


def _build_kernel(single=False):
    nc = bacc.Bacc(
        "TRN2", target_bir_lowering=False, debug=False,
        enable_asserts=False, num_devices=1 if single else NCORES,
    )
    # ---------------- per-core external I/O ----------------
    u_d = nc.dram_tensor("u", [LC, H], F32R, kind="ExternalInput").ap()
    cs_d = nc.dram_tensor("cs", [P, T], BF16, kind="ExternalInput").ap()
    sn_d = nc.dram_tensor("sn", [P, T], BF16, kind="ExternalInput").ap()
    rrep_d = nc.dram_tensor("rrep", [P, T], F32, kind="ExternalInput").ap()
    rpow_d = nc.dram_tensor("rpow", [P, T], BF16, kind="ExternalInput").ap()
    btr_d = nc.dram_tensor("btr", [2, P, P], F32R, kind="ExternalInput").ap()
    bti_d = nc.dram_tensor("bti", [2, P, P], F32R, kind="ExternalInput").ap()
    crt_d = nc.dram_tensor("crt", [P, H], BF16, kind="ExternalInput").ap()
    cit_d = nc.dram_tensor("cit", [P, H], BF16, kind="ExternalInput").ap()
    dd_d = nc.dram_tensor("dd", [2, P, H], F32R, kind="ExternalInput").ap()
    mc_d = nc.dram_tensor("mc", [P, 2 * NCH], F32, kind="ExternalInput").ap()
    ctc_d = nc.dram_tensor("ctc", [P, 2], F32, kind="ExternalInput").ap()
    wgr_d = nc.dram_tensor("wgr", [P, NCORES], F32, kind="ExternalInput").ap()
    wgi_d = nc.dram_tensor("wgi", [P, NCORES], F32, kind="ExternalInput").ap()
    id_d = nc.dram_tensor("ident", [P, P], F32R, kind="ExternalInput").ap()
    out_d = nc.dram_tensor("out", [LC, H], F32, kind="ExternalOutput").ap()

    with tile.TileContext(nc) as tc:
        _body(tc, nc, u_d, cs_d, sn_d, rrep_d, rpow_d, btr_d, bti_d,
              crt_d, cit_d, dd_d, mc_d, ctc_d, wgr_d, wgi_d, id_d, out_d,
              single=single)
    nc.compile()
    return nc


def _body(tc, nc, u_d, cs_d, sn_d, rrep_d, rpow_d, btr_d, bti_d,
          crt_d, cit_d, dd_d, mc_d, ctc_d, wgr_d, wgi_d, id_d, out_d,
          single=False):
    with (
        tc.tile_pool(name="const", bufs=1) as cpool,
        tc.tile_pool(name="span", bufs=1) as spool,
        tc.tile_pool(name="work", bufs=3) as wpool,
        tc.tile_pool(name="dram", bufs=1, space="DRAM") as dpool,
    ):
        # ---- constants ----
        cs_sb = cpool.tile([P, T], BF16)
        sn_sb = cpool.tile([P, T], BF16)
        rrep_sb = cpool.tile([P, T], F32)
        rpow_sb = cpool.tile([P, T], BF16)
        btr_sb = cpool.tile([P, 2, P], F32R)
        bti_sb = cpool.tile([P, 2, P], F32R)
        crt_sb = cpool.tile([P, H], BF16)
        cit_sb = cpool.tile([P, H], BF16)
        dd_sb = cpool.tile([P, 2, H], F32R)
        mc_sb = cpool.tile([P, 2 * NCH], F32)
        ctc_sb = cpool.tile([P, 2], F32)
        wgr_sb = cpool.tile([P, NCORES], F32)
        wgi_sb = cpool.tile([P, NCORES], F32)
        ident = cpool.tile([P, P], F32R)
        nc.sync.dma_start(ident[:], id_d)
        for dst, src in ((cs_sb, cs_d), (sn_sb, sn_d), (rrep_sb, rrep_d),
                         (rpow_sb, rpow_d), (crt_sb, crt_d), (cit_sb, cit_d),
                         (mc_sb, mc_d), (ctc_sb, ctc_d),
                         (wgr_sb, wgr_d), (wgi_sb, wgi_d)):
            nc.sync.dma_start(dst[:], src)
        for dst, src in ((btr_sb, btr_d), (bti_sb, bti_d), (dd_sb, dd_d)):
            for a in range(2):
                nc.sync.dma_start(dst[:, a, :], src[a])

        # ---- span-persistent state ----
        ut_sb = spool.tile([P, 2, LC], F32R)      # u^T, built on-chip
        yr_sb = spool.tile([P, LC], BF16)         # scan outputs (rotated basis)
        yi_sb = spool.tile([P, LC], BF16)
        gcols = spool.tile([P, 2 * (NCH + 1)], F32)   # chunk-carry columns

        # =============== phase 1: transpose u, Bu, rotate, scan ===============
        with (
            tc.tile_pool(name="psum", bufs=2, space="PSUM") as ppool,
            tc.tile_pool(name="tpsum", bufs=2, space="PSUM") as tpool,
        ):
            for c in range(NCH):
                t0 = c * T
                # on-chip transpose: u[t0:t0+T, :] -> ut[:, :, t0:t0+T]
                for k in range(T // P):
                    r0 = t0 + k * P
                    un = wpool.tile([P, H], F32R, tag="un")
                    nc.sync.dma_start(un[:], u_d[r0:r0 + P, :])
                    for hh in range(2):
                        pt = tpool.tile([P, P], F32R, tag=f"pt{hh}")
                        nc.tensor.transpose(pt[:], un[:, hh * P:(hh + 1) * P],
                                            ident[:])
                        nc.scalar.copy(ut_sb[:, hh, r0:r0 + P], pt[:])
                pbr = ppool.tile([P, T], F32, tag="pbur")
                pbi = ppool.tile([P, T], F32, tag="pbui")
                for a in range(2):
                    nc.tensor.matmul(pbr[:], btr_sb[:, a, :],
                                     ut_sb[:, a, t0:t0 + T],
                                     start=(a == 0), stop=(a == 1))
                for a in range(2):
                    nc.tensor.matmul(pbi[:], bti_sb[:, a, :],
                                     ut_sb[:, a, t0:t0 + T],
                                     start=(a == 0), stop=(a == 1))
                bur = wpool.tile([P, T], BF16, tag="bur")
                bui = wpool.tile([P, T], BF16, tag="bui")
                nc.scalar.copy(bur[:], pbr[:])
                nc.scalar.copy(bui[:], pbi[:])
                # w = e^{-i th s} * Bu
                m1 = wpool.tile([P, T], BF16, tag="m1")
                m2 = wpool.tile([P, T], BF16, tag="m2")
                m3 = wpool.tile([P, T], BF16, tag="m3")
                m4 = wpool.tile([P, T], BF16, tag="m4")
                wr = wpool.tile([P, T], BF16, tag="wr")
                wi = wpool.tile([P, T], BF16, tag="wi")
                nc.vector.tensor_mul(m1[:], cs_sb[:], bur[:])
                nc.gpsimd.tensor_mul(m2[:], sn_sb[:], bui[:])
                nc.vector.tensor_add(wr[:], m1[:], m2[:])
                nc.vector.tensor_mul(m3[:], cs_sb[:], bui[:])
                nc.gpsimd.tensor_mul(m4[:], sn_sb[:], bur[:])
                nc.vector.tensor_sub(wi[:], m3[:], m4[:])
                # scans
                init_r = 0.0 if c == 0 else gcols[:, 2 * c:2 * c + 1]
                init_i = 0.0 if c == 0 else gcols[:, 2 * c + 1:2 * c + 2]
                nc.vector.tensor_tensor_scan(
                    yr_sb[:, t0:t0 + T], rrep_sb[:], wr[:], init_r,
                    op0=MUL, op1=ADD)
                nc.vector.tensor_tensor_scan(
                    yi_sb[:, t0:t0 + T], rrep_sb[:], wi[:], init_i,
                    op0=MUL, op1=ADD)
                # chunk carry: g_{c+1} = e^{i th T} * y[:, last]
                if c < NCH - 1:
                    ylr = yr_sb[:, t0 + T - 1:t0 + T]
                    yli = yi_sb[:, t0 + T - 1:t0 + T]
                    tc1 = wpool.tile([P, 1], F32, tag="tc1")
                    tc2 = wpool.tile([P, 1], F32, tag="tc2")
                    nc.vector.tensor_scalar_mul(tc1[:], yli[:], ctc_sb[:, 1:2])
                    nc.vector.scalar_tensor_tensor(
                        gcols[:, 2 * c + 2:2 * c + 3], ylr, ctc_sb[:, 0:1],
                        tc1[:], op0=MUL, op1=mybir.AluOpType.subtract)
                    nc.vector.tensor_scalar_mul(tc2[:], ylr[:], ctc_sb[:, 1:2])
                    nc.vector.scalar_tensor_tensor(
                        gcols[:, 2 * c + 3:2 * c + 4], yli, ctc_sb[:, 0:1],
                        tc2[:], op0=MUL, op1=ADD)

        # =============== cross-core carry exchange ===============
        e_loc = dpool.tile([P, 2], F32)
        e_all = dpool.tile([NCORES * P, 2], F32)
        e_sb = cpool.tile([P, 2], F32)
        nc.vector.tensor_copy(e_sb[:, 0:1], yr_sb[:, LC - 1:LC])
        nc.vector.tensor_copy(e_sb[:, 1:2], yi_sb[:, LC - 1:LC])
        nc.gpsimd.dma_start(e_loc[:], e_sb[:])
        if single:
            nc.gpsimd.dma_start(e_all[0:P, :], e_loc[:])
        else:
            nc.gpsimd.collective_compute(
                "AllGather", mybir.AluOpType.bypass,
                replica_groups=[list(range(NCORES))],
                ins=[e_loc.opt()], outs=[e_all.opt()])
        eall_sb = cpool.tile([P, 2 * NCORES], F32)
        for j in range(NCORES):
            nc.gpsimd.dma_start(eall_sb[:, 2 * j:2 * j + 2],
                                e_all[j * P:(j + 1) * P, :])
        # Cin = sum_j W'_j * E_j   (complex, W' host-precomputed per core)
        er_v = eall_sb[:, 0:2 * NCORES:2]
        ei_v = eall_sb[:, 1:2 * NCORES:2]
        ta = cpool.tile([P, NCORES], F32)
        tb = cpool.tile([P, NCORES], F32)
        cin = cpool.tile([P, 2], F32)
        nc.vector.tensor_mul(ta[:], wgr_sb[:], er_v)
        nc.vector.tensor_mul(tb[:], wgi_sb[:], ei_v)
        nc.vector.tensor_sub(ta[:], ta[:], tb[:])
        nc.vector.tensor_reduce(cin[:, 0:1], ta[:], axis=AX, op=ADD)
        nc.vector.tensor_mul(ta[:], wgi_sb[:], er_v)
        nc.vector.tensor_mul(tb[:], wgr_sb[:], ei_v)
        nc.vector.tensor_add(ta[:], ta[:], tb[:])
        nc.vector.tensor_reduce(cin[:, 1:2], ta[:], axis=AX, op=ADD)
        # G_all[:, c] = (e^{i th} A^{T c}) * Cin  for each chunk c
        gr_all = cpool.tile([P, NCH], F32)
        gi_all = cpool.tile([P, NCH], F32)
        tg = cpool.tile([P, NCH], F32)
        nc.vector.tensor_scalar_mul(gr_all[:], mc_sb[:, 0:NCH], cin[:, 0:1])
        nc.vector.tensor_scalar_mul(tg[:], mc_sb[:, NCH:2 * NCH], cin[:, 1:2])
        nc.vector.tensor_sub(gr_all[:], gr_all[:], tg[:])
        nc.vector.tensor_scalar_mul(gi_all[:], mc_sb[:, NCH:2 * NCH], cin[:, 0:1])
        nc.vector.tensor_scalar_mul(tg[:], mc_sb[:, 0:NCH], cin[:, 1:2])
        nc.vector.tensor_add(gi_all[:], gi_all[:], tg[:])

        # =============== phase 2: correct, unrotate, project ===============
        with tc.tile_pool(name="opsum", bufs=2, space="PSUM") as opool:
            for c in range(NCH):
                t0 = c * T
                # y += r^{s+1} * G_c
                cr1 = wpool.tile([P, T], BF16, tag="cr1")
                cr2 = wpool.tile([P, T], BF16, tag="cr2")
                yrc = wpool.tile([P, T], BF16, tag="yrc")
                yic = wpool.tile([P, T], BF16, tag="yic")
                nc.scalar.mul(cr1[:], rpow_sb[:], gr_all[:, c:c + 1])
                nc.scalar.mul(cr2[:], rpow_sb[:], gi_all[:, c:c + 1])
                nc.vector.tensor_add(yrc[:], cr1[:], yr_sb[:, t0:t0 + T])
                nc.vector.tensor_add(yic[:], cr2[:], yi_sb[:, t0:t0 + T])
                # x = e^{+i th s} * y
                n1 = wpool.tile([P, T], BF16, tag="n1")
                n2 = wpool.tile([P, T], BF16, tag="n2")
                xr = wpool.tile([P, T], BF16, tag="xr")
                xi = wpool.tile([P, T], BF16, tag="xi")
                nc.vector.tensor_mul(n1[:], cs_sb[:], yrc[:])
                nc.gpsimd.tensor_mul(n2[:], sn_sb[:], yic[:])
                nc.vector.tensor_sub(xr[:], n1[:], n2[:])
                nc.gpsimd.tensor_mul(n1[:], cs_sb[:], yic[:])
                nc.vector.tensor_mul(n2[:], sn_sb[:], yrc[:])
                nc.vector.tensor_add(xi[:], n1[:], n2[:])
                # ys^T-free output: ys[t,h] = 2Re(C x)[t,h] + (D u)[t,h]
                po = opool.tile([P, 4, H], F32, tag="po")
                ob = wpool.tile([P, 4, H], F32, tag="ob")
                for a in range(4):
                    sl = slice(a * P, (a + 1) * P)
                    nc.tensor.matmul(po[:, a, :], xr[:, sl], crt_sb[:],
                                     start=True, stop=False)
                    nc.tensor.matmul(po[:, a, :], xi[:, sl], cit_sb[:],
                                     start=False, stop=False)
                    nc.tensor.matmul(po[:, a, :],
                                     ut_sb[:, 0, t0 + a * P:t0 + (a + 1) * P],
                                     dd_sb[:, 0, :], start=False, stop=False)
                    nc.tensor.matmul(po[:, a, :],
                                     ut_sb[:, 1, t0 + a * P:t0 + (a + 1) * P],
                                     dd_sb[:, 1, :], start=False, stop=True)
                    nc.scalar.copy(ob[:, a, :], po[:, a, :])
                    nc.sync.dma_start(out_d[t0 + a * P:t0 + (a + 1) * P, :],
                                      ob[:, a, :])


# ======================= host side =======================

_STATE = {}


def _param_tables(Lambda_re, Lambda_im, B, C, D, log_step):
    """f64 host-side parameter/table preparation -> GLOBAL (concat over
    cores along axis 0) input arrays for every tensor except `u`."""
    Lam = Lambda_re.astype(np.float64) + 1j * Lambda_im.astype(np.float64)
    step = np.exp(log_step.astype(np.float64))
    A = np.exp(Lam * step)                        # (P,)
    r = np.abs(A)
    th = np.imag(Lam * step)
    Bt = B[..., 0].astype(np.float64) + 1j * B[..., 1].astype(np.float64)
    Bbar = ((A - 1.0) / Lam)[:, None] * Bt        # (P, H)
    Ct = C[..., 0].astype(np.float64) + 1j * C[..., 1].astype(np.float64)

    s = np.arange(T, dtype=np.float64)
    cs = np.cos(th[:, None] * s[None, :])
    sn = np.sin(th[:, None] * s[None, :])
    rrep = np.broadcast_to(r[:, None], (P, T))
    rpow = r[:, None] ** (s[None, :] + 1.0)

    Br, Bi = np.real(Bbar), np.imag(Bbar)
    btr = np.stack([Br.T[a * P:(a + 1) * P] for a in range(2)])   # (2,128h,128p)
    bti = np.stack([Bi.T[a * P:(a + 1) * P] for a in range(2)])
    crt = 2.0 * np.real(Ct).T                                     # (P, H)
    cit = -2.0 * np.imag(Ct).T
    dd = np.zeros((2, P, H), np.float64)
    for a in range(2):
        for hh in range(P):
            dd[a, hh, a * P + hh] = D[a * P + hh]
    # per-chunk carry application matrices  e^{i th} A^{T c}
    mc = np.zeros((P, 2 * NCH), np.float64)
    eA = np.exp(1j * th)
    for c in range(NCH):
        m = eA * (A ** (T * c))
        mc[:, c] = np.real(m)
        mc[:, NCH + c] = np.imag(m)
    ctc = np.stack([np.cos(th * T), np.sin(th * T)], axis=1)      # (P, 2)

    ALC = A ** LC
    eE = np.exp(1j * th * (T - 1))     # local y -> span-final state phase

    def rep(x):
        # identical per core -> global concat along axis 0
        return np.concatenate([x] * NCORES, axis=0)

    wgr = np.zeros((NCORES, P, NCORES), np.float64)
    wgi = np.zeros((NCORES, P, NCORES), np.float64)
    for i in range(NCORES):
        for j in range(i):
            w = (ALC ** (i - 1 - j)) * eE
            wgr[i, :, j] = np.real(w)
            wgi[i, :, j] = np.imag(w)

    return {
        "cs": rep(cs.astype(NPBF16)),
        "sn": rep(sn.astype(NPBF16)),
        "rrep": rep(np.ascontiguousarray(rrep).astype(np.float32)),
        "rpow": rep(rpow.astype(NPBF16)),
        "btr": rep(btr.astype(np.float32)),
        "bti": rep(bti.astype(np.float32)),
        "crt": rep(crt.astype(NPBF16)),
        "cit": rep(cit.astype(NPBF16)),
        "dd": rep(dd.astype(np.float32)),
        "mc": rep(mc.astype(np.float32)),
        "ctc": rep(ctc.astype(np.float32)),
        "wgr": wgr.reshape(NCORES * P, NCORES).astype(np.float32),
        "wgi": wgi.reshape(NCORES * P, NCORES).astype(np.float32),
        "ident": rep(np.eye(P, dtype=np.float32)),
    }


def _in_maps_for_sim(inputs):
    """Per-core input maps (for MultiCoreSim in test.py)."""
    tabs = _param_tables(inputs["Lambda_re"], inputs["Lambda_im"],
                         inputs["B"], inputs["C"], inputs["D"],
                         inputs["log_step"])
    u = np.ascontiguousarray(np.asarray(inputs["input_sequence"],
                                        dtype=np.float32))
    maps = []
    for i in range(NCORES):
        m = {"u": u[i * LC:(i + 1) * LC]}
        for name, g in tabs.items():
            per = g.shape[0] // NCORES
            m[name] = g[i * per:(i + 1) * per]
        maps.append(m)
    return maps


def _build_dispatch(nc):
    """Hoisted equivalent of concourse.bass2jax.run_bass_via_pjrt: build
    the shard_map'd jit ONCE so steady-state calls skip retrace/relower."""
    install_neuronx_cc_hook()
    assert nc.dbg_addr is None and not nc.dbg_callbacks
    partition_name = (nc.partition_id_tensor.name
                      if nc.partition_id_tensor else None)

    in_names, out_names, out_avals = [], [], []
    for alloc in nc.m.functions[0].allocations:
        if not isinstance(alloc, mybir.MemoryLocationSet):
            continue
        assert alloc.memorylocations
        name = alloc.memorylocations[0].name
        if alloc.kind == "ExternalInput":
            if name != partition_name:
                in_names.append(name)
        elif alloc.kind == "ExternalOutput":
            assert alloc.tensor_shape is not None and alloc.dtype is not None
            out_names.append(name)
            shape = tuple(alloc.tensor_shape)
            dtype = mybir.dt.np(alloc.dtype)
            out_avals.append(jax.core.ShapedArray(shape, dtype))
    n_params = len(in_names)
    n_outs = len(out_avals)
    all_names = list(in_names) + list(out_names)
    if partition_name is not None:
        all_names.append(partition_name)

    def _kernel_body(*args):
        operands = list(args)
        if partition_name is not None:
            operands.append(partition_id_tensor())
        outs = _bass_exec_p.bind(
            *operands,
            out_avals=tuple(out_avals),
            in_names=tuple(all_names),
            out_names=tuple(out_names),
            lowering_input_output_aliases=(),
            sim_require_finite=True,
            sim_require_nnan=True,
            nc=nc,
        )
        return tuple(outs)

    devices = jax.devices()[:NCORES]
    assert len(devices) == NCORES
    mesh = Mesh(np.asarray(devices), ("core",))
    in_specs = (PartitionSpec("core"),) * (n_params + n_outs)
    out_specs = (PartitionSpec("core"),) * n_outs
    donate = tuple(range(n_params, n_params + n_outs))
    fn = jax.jit(
        shard_map(_kernel_body, mesh=mesh, in_specs=in_specs,
                  out_specs=out_specs, check_rep=False),
        donate_argnums=donate, keep_unused=True,
    )
    return fn, in_names, mesh


def _get_state():
    if "fn" not in _STATE:
        nc = _build_kernel()
        fn, in_names, mesh = _build_dispatch(nc)
        _STATE.update(nc=nc, fn=fn, in_names=in_names, mesh=mesh,
                      sharding=NamedSharding(mesh, PartitionSpec("core")))
    return _STATE


def kernel(Lambda_re, Lambda_im, B, C, D, log_step, input_sequence):
    st = _get_state()
    pkey = (np.asarray(Lambda_re).tobytes(), np.asarray(Lambda_im).tobytes(),
            np.asarray(B).tobytes(), np.asarray(C).tobytes(),
            np.asarray(D).tobytes(), np.asarray(log_step).tobytes())
    if st.get("pkey") != pkey:
        tabs = _param_tables(np.asarray(Lambda_re), np.asarray(Lambda_im),
                             np.asarray(B), np.asarray(C), np.asarray(D),
                             np.asarray(log_step))
        st["const_dev"] = {k: jax.device_put(v, st["sharding"])
                           for k, v in tabs.items()}
        st["pkey"] = pkey
        st["prev_out"] = None

    u = np.ascontiguousarray(np.asarray(input_sequence, dtype=np.float32))
    args = [u if name == "u" else st["const_dev"][name]
            for name in st["in_names"]]
    out_buf = st["prev_out"]
    if out_buf is None:
        out_buf = np.zeros((L, H), np.float32)
    out_arrs = st["fn"](*args, out_buf)
    out_dev = out_arrs[0]
    host = np.asarray(out_dev)
    st["prev_out"] = out_dev
    return host


# revision 17
# speedup vs baseline: 1.2749x; 1.2749x over previous
"""Trainium2 Bass kernel for a continuous-time diagonal SSM layer (S5/S4D-style).

Math (see reference):
    A = exp(Lambda * step)                 (P,) complex, |A| = r, arg = theta
    Bu[t] = B_bar @ u[t]                   (L, P) complex
    x[t]  = A * x[t-1] + Bu[t]             diagonal complex scan
    ys    = 2 Re(C_tilde @ x) + D * u

Kernel strategy (8 cores, sequence-parallel over L):
  * core i owns a contiguous span of LC = L/8 timesteps, chunked by T=512.
  * rotation trick: with A = r e^{i.th}, define y[t] = e^{-i.th.t} x[t].
    Then y obeys a REAL per-partition recurrence y[t] = r y[t-1] + w[t],
    w[t] = e^{-i.th.t} Bu[t], which maps onto the DVE tensor_tensor_scan
    instruction (state = data0*state + data1 along the free dim).
  * layout: state-major [128 modes (partitions) x time (free)].  u arrives
    in natural (LC, H) layout and is transposed on-chip via PE-transpose
    (identity matmul) into u^T tiles; Bu^T comes from PE matmuls vs u^T;
    the output matmul produces time-major ys directly (lhsT = x tiles),
    D*u is folded in as a diagonal matmul from u^T.  Phase tables
    (cos/sin of th*s, s in [0,T)) are chunk-local, host-precomputed in
    f64; chunk carries chain through the scan `initial` operand after a
    tiny per-chunk basis rotation.
  * cross-core carry: each core AllGathers its span-final scan state (1KB),
    combines them with host-precomputed decay weights, and adds the
    correction r^{s+1}*G into y before the output projection.
  * dtypes: bf16 elementwise domain, f32 scan multiplier (bf16 cannot
    represent slow decay rates like r=0.99995), float32r matmuls.

Host strategy (the per-call wall-clock is dominated by host work, not the
~100us device program):
  * the jax/PJRT dispatch (shard_map over 8 cores -> bass_exec custom
    call) is built ONCE and cached; steady-state calls hit the jit C++
    fast path instead of re-tracing + re-lowering per call.
  * all parameter-derived tables are cached keyed by the raw param bytes
    and kept device-resident (device_put once); only `u` (the activation)
    crosses host->device per call, with no host-side reshape/transpose
    (the global input IS input_sequence).
  * the previous call's device-resident output is donated as the next
    call's output buffer, so no zero-buffer upload per call.
"""

import numpy as np
import ml_dtypes
import jax
from jax.experimental.shard_map import shard_map
from jax.sharding import Mesh, NamedSharding, PartitionSpec

import concourse.bass as bass
import concourse.bacc as bacc
import concourse.tile as tile
import concourse.mybir as mybir
from concourse.bass2jax import (
    _bass_exec_p,
    install_neuronx_cc_hook,
    partition_id_tensor,
)

F32 = mybir.dt.float32
F32R = mybir.dt.float32r
BF16 = mybir.dt.bfloat16
NPBF16 = ml_dtypes.bfloat16
AX = mybir.AxisListType.X
MUL = mybir.AluOpType.mult
ADD = mybir.AluOpType.add

L, H, P = 32768, 256, 128
NCORES = 8
LC = L // NCORES          # 4096 timesteps per core
T = 512                   # chunk length
NCH = LC // T             # 8 chunks per core
KT = T // P               # 4 row-tiles of u per chunk

# packed-constant layouts (free-dim offsets)
CBF_W = 3 * T + 2 * H     # cs | sn | rpow | crt | cit
CF32_W = T + 2 * NCH + 2 + 2 * NCORES   # rrep | mc | ctc | wgr | wgi
CFR_W = 4 * P + 2 * H + P               # btr(2) | bti(2) | dd(2) | ident


def _build_kernel(single=False):
    nc = bacc.Bacc(
        "TRN2", target_bir_lowering=False, debug=False,
        enable_asserts=False, num_devices=1 if single else NCORES,
    )
    # ---------------- per-core external I/O ----------------
    # u in natural layout, viewed as [row-tiles, P, H] for permuted DMA
    u_d = nc.dram_tensor("u", [LC // P, P, H], F32R, kind="ExternalInput").ap()
    # constants packed by dtype into single tensors (one DMA each)
    cbf_d = nc.dram_tensor("cbf", [P, CBF_W], BF16, kind="ExternalInput").ap()
    cf32_d = nc.dram_tensor("cf32", [P, CF32_W], F32, kind="ExternalInput").ap()
    cfr_d = nc.dram_tensor("cfr", [P, CFR_W], F32R, kind="ExternalInput").ap()
    out_d = nc.dram_tensor("out", [NCH, T // P, P, H], F32,
                           kind="ExternalOutput").ap()

    with tile.TileContext(nc) as tc:
        _body(tc, nc, u_d, cbf_d, cf32_d, cfr_d, out_d, single=single)
    nc.compile()
    return nc


def _body(tc, nc, u_d, cbf_d, cf32_d, cfr_d, out_d, single=False):
    with (
        tc.tile_pool(name="const", bufs=1) as cpool,
        tc.tile_pool(name="span", bufs=1) as spool,
        tc.tile_pool(name="work", bufs=3) as wpool,
        tc.tile_pool(name="dram", bufs=1, space="DRAM") as dpool,
    ):
        # ---- packed constants: one DMA per dtype group ----
        cbf = cpool.tile([P, CBF_W], BF16)
        cf32 = cpool.tile([P, CF32_W], F32)
        cfr = cpool.tile([P, CFR_W], F32R)
        nc.sync.dma_start(cbf[:], cbf_d)
        nc.sync.dma_start(cf32[:], cf32_d)
        nc.sync.dma_start(cfr[:], cfr_d)
        cs_sb = cbf[:, 0:T]
        sn_sb = cbf[:, T:2 * T]
        rpow_sb = cbf[:, 2 * T:3 * T]
        crt_sb = cbf[:, 3 * T:3 * T + H]
        cit_sb = cbf[:, 3 * T + H:3 * T + 2 * H]
        rrep_sb = cf32[:, 0:T]
        mc_sb = cf32[:, T:T + 2 * NCH]
        ctc_sb = cf32[:, T + 2 * NCH:T + 2 * NCH + 2]
        wgr_sb = cf32[:, T + 2 * NCH + 2:T + 2 * NCH + 2 + NCORES]
        wgi_sb = cf32[:, T + 2 * NCH + 2 + NCORES:T + 2 * NCH + 2 + 2 * NCORES]
        btr_sb = [cfr[:, a * P:(a + 1) * P] for a in range(2)]
        bti_sb = [cfr[:, 2 * P + a * P:2 * P + (a + 1) * P] for a in range(2)]
        dd_sb = [cfr[:, 4 * P + a * H:4 * P + (a + 1) * H] for a in range(2)]
        ident = cfr[:, 4 * P + 2 * H:4 * P + 2 * H + P]

        # ---- span-persistent state ----
        ut_sb = spool.tile([P, 2, LC], F32R)      # u^T, built on-chip
        yr_sb = spool.tile([P, LC], BF16)         # scan outputs (rotated basis)
        yi_sb = spool.tile([P, LC], BF16)
        gcols = spool.tile([P, 2 * (NCH + 1)], F32)   # chunk-carry columns

        # =============== phase 1: transpose u, Bu, rotate, scan ===============
        with (
            tc.tile_pool(name="psum", bufs=2, space="PSUM") as ppool,
            tc.tile_pool(name="tpsum", bufs=2, space="PSUM") as tpool,
        ):
            for c in range(NCH):
                t0 = c * T
                # one permuted DMA brings u[t0:t0+T, :] as [P, KT, H]
                un4 = wpool.tile([P, KT, H], F32R, tag="un")
                nc.sync.dma_start(un4[:],
                                  u_d[c * KT:(c + 1) * KT].transpose([1, 0, 2]))
                # PE-transpose into one PSUM bank per half, single copy out
                for hh in range(2):
                    pt4 = tpool.tile([P, T], F32R, tag=f"pt{hh}")
                    for k in range(KT):
                        nc.tensor.transpose(
                            pt4[:, k * P:(k + 1) * P],
                            un4[:, k, hh * P:(hh + 1) * P], ident)
                    nc.scalar.copy(ut_sb[:, hh, t0:t0 + T], pt4[:])
                pbr = ppool.tile([P, T], F32, tag="pbur")
                pbi = ppool.tile([P, T], F32, tag="pbui")
                for a in range(2):
                    nc.tensor.matmul(pbr[:], btr_sb[a],
                                     ut_sb[:, a, t0:t0 + T],
                                     start=(a == 0), stop=(a == 1))
                for a in range(2):
                    nc.tensor.matmul(pbi[:], bti_sb[a],
                                     ut_sb[:, a, t0:t0 + T],
                                     start=(a == 0), stop=(a == 1))
                bur = wpool.tile([P, T], BF16, tag="bur")
                bui = wpool.tile([P, T], BF16, tag="bui")
                nc.scalar.copy(bur[:], pbr[:])
                nc.scalar.copy(bui[:], pbi[:])
                # w = e^{-i th s} * Bu
                m1 = wpool.tile([P, T], BF16, tag="m1")
                m2 = wpool.tile([P, T], BF16, tag="m2")
                m3 = wpool.tile([P, T], BF16, tag="m3")
                m4 = wpool.tile([P, T], BF16, tag="m4")
                wr = wpool.tile([P, T], BF16, tag="wr")
                wi = wpool.tile([P, T], BF16, tag="wi")
                nc.vector.tensor_mul(m1[:], cs_sb, bur[:])
                nc.gpsimd.tensor_mul(m2[:], sn_sb, bui[:])
                nc.vector.tensor_add(wr[:], m1[:], m2[:])
                nc.vector.tensor_mul(m3[:], cs_sb, bui[:])
                nc.gpsimd.tensor_mul(m4[:], sn_sb, bur[:])
                nc.vector.tensor_sub(wi[:], m3[:], m4[:])
                # scans
                init_r = 0.0 if c == 0 else gcols[:, 2 * c:2 * c + 1]
                init_i = 0.0 if c == 0 else gcols[:, 2 * c + 1:2 * c + 2]
                nc.vector.tensor_tensor_scan(
                    yr_sb[:, t0:t0 + T], rrep_sb, wr[:], init_r,
                    op0=MUL, op1=ADD)
                nc.vector.tensor_tensor_scan(
                    yi_sb[:, t0:t0 + T], rrep_sb, wi[:], init_i,
                    op0=MUL, op1=ADD)
                # chunk carry: g_{c+1} = e^{i th T} * y[:, last]
                if c < NCH - 1:
                    ylr = yr_sb[:, t0 + T - 1:t0 + T]
                    yli = yi_sb[:, t0 + T - 1:t0 + T]
                    tc1 = wpool.tile([P, 1], F32, tag="tc1")
                    tc2 = wpool.tile([P, 1], F32, tag="tc2")
                    nc.vector.tensor_scalar_mul(tc1[:], yli[:], ctc_sb[:, 1:2])
                    nc.vector.scalar_tensor_tensor(
                        gcols[:, 2 * c + 2:2 * c + 3], ylr, ctc_sb[:, 0:1],
                        tc1[:], op0=MUL, op1=mybir.AluOpType.subtract)
                    nc.vector.tensor_scalar_mul(tc2[:], ylr[:], ctc_sb[:, 1:2])
                    nc.vector.scalar_tensor_tensor(
                        gcols[:, 2 * c + 3:2 * c + 4], yli, ctc_sb[:, 0:1],
                        tc2[:], op0=MUL, op1=ADD)

        # =============== cross-core carry exchange ===============
        e_loc = dpool.tile([P, 2], F32)
        e_all = dpool.tile([NCORES, P, 2], F32)
        e_sb = cpool.tile([P, 2], F32)
        nc.vector.tensor_copy(e_sb[:, 0:1], yr_sb[:, LC - 1:LC])
        nc.vector.tensor_copy(e_sb[:, 1:2], yi_sb[:, LC - 1:LC])
        nc.gpsimd.dma_start(e_loc[:], e_sb[:])
        if single:
            nc.gpsimd.dma_start(e_all[0], e_loc[:])
        else:
            nc.gpsimd.collective_compute(
                "AllGather", mybir.AluOpType.bypass,
                replica_groups=[list(range(NCORES))],
                ins=[e_loc.opt()], outs=[e_all.opt()])
        # one permuted DMA gathers all spans' finals as [P, NCORES, 2]
        eall_sb = cpool.tile([P, NCORES, 2], F32)
        nc.sync.dma_start(eall_sb[:], e_all.transpose([1, 0, 2]))
        # Cin = sum_j W'_j * E_j   (complex, W' host-precomputed per core)
        er_v = eall_sb[:, :, 0]
        ei_v = eall_sb[:, :, 1]
        ta = cpool.tile([P, NCORES], F32)
        tb = cpool.tile([P, NCORES], F32)
        cin = cpool.tile([P, 2], F32)
        nc.vector.tensor_mul(ta[:], wgr_sb, er_v)
        nc.vector.tensor_mul(tb[:], wgi_sb, ei_v)
        nc.vector.tensor_sub(ta[:], ta[:], tb[:])
        nc.vector.tensor_reduce(cin[:, 0:1], ta[:], axis=AX, op=ADD)
        nc.vector.tensor_mul(ta[:], wgi_sb, er_v)
        nc.vector.tensor_mul(tb[:], wgr_sb, ei_v)
        nc.vector.tensor_add(ta[:], ta[:], tb[:])
        nc.vector.tensor_reduce(cin[:, 1:2], ta[:], axis=AX, op=ADD)
        # G_all[:, c] = (e^{i th} A^{T c}) * Cin  for each chunk c
        gr_all = cpool.tile([P, NCH], F32)
        gi_all = cpool.tile([P, NCH], F32)
        tg = cpool.tile([P, NCH], F32)
        nc.vector.tensor_scalar_mul(gr_all[:], mc_sb[:, 0:NCH], cin[:, 0:1])
        nc.vector.tensor_scalar_mul(tg[:], mc_sb[:, NCH:2 * NCH], cin[:, 1:2])
        nc.vector.tensor_sub(gr_all[:], gr_all[:], tg[:])
        nc.vector.tensor_scalar_mul(gi_all[:], mc_sb[:, NCH:2 * NCH], cin[:, 0:1])
        nc.vector.tensor_scalar_mul(tg[:], mc_sb[:, 0:NCH], cin[:, 1:2])
        nc.vector.tensor_add(gi_all[:], gi_all[:], tg[:])

        # =============== phase 2: correct, unrotate, project ===============
        with tc.tile_pool(name="opsum", bufs=2, space="PSUM") as opool:
            for c in range(NCH):
                t0 = c * T
                # y += r^{s+1} * G_c
                cr1 = wpool.tile([P, T], BF16, tag="cr1")
                cr2 = wpool.tile([P, T], BF16, tag="cr2")
                yrc = wpool.tile([P, T], BF16, tag="yrc")
                yic = wpool.tile([P, T], BF16, tag="yic")
                nc.scalar.mul(cr1[:], rpow_sb, gr_all[:, c:c + 1])
                nc.scalar.mul(cr2[:], rpow_sb, gi_all[:, c:c + 1])
                nc.vector.tensor_add(yrc[:], cr1[:], yr_sb[:, t0:t0 + T])
                nc.vector.tensor_add(yic[:], cr2[:], yi_sb[:, t0:t0 + T])
                # x = e^{+i th s} * y
                n1 = wpool.tile([P, T], BF16, tag="n1")
                n2 = wpool.tile([P, T], BF16, tag="n2")
                xr = wpool.tile([P, T], BF16, tag="xr")
                xi = wpool.tile([P, T], BF16, tag="xi")
                nc.vector.tensor_mul(n1[:], cs_sb, yrc[:])
                nc.gpsimd.tensor_mul(n2[:], sn_sb, yic[:])
                nc.vector.tensor_sub(xr[:], n1[:], n2[:])
                nc.gpsimd.tensor_mul(n1[:], cs_sb, yic[:])
                nc.vector.tensor_mul(n2[:], sn_sb, yrc[:])
                nc.vector.tensor_add(xi[:], n1[:], n2[:])
                # ys^T-free output: ys[t,h] = 2Re(C x)[t,h] + (D u)[t,h]
                po = opool.tile([P, 4, H], F32, tag="po")
                ob = wpool.tile([P, 4, H], F32, tag="ob")
                for a in range(4):
                    sl = slice(a * P, (a + 1) * P)
                    nc.tensor.matmul(po[:, a, :], xr[:, sl], crt_sb,
                                     start=True, stop=False)
                    nc.tensor.matmul(po[:, a, :], xi[:, sl], cit_sb,
                                     start=False, stop=False)
                    nc.tensor.matmul(po[:, a, :],
                                     ut_sb[:, 0, t0 + a * P:t0 + (a + 1) * P],
                                     dd_sb[0], start=False, stop=False)
                    nc.tensor.matmul(po[:, a, :],
                                     ut_sb[:, 1, t0 + a * P:t0 + (a + 1) * P],
                                     dd_sb[1], start=False, stop=True)
                nc.scalar.copy(ob[:], po[:])
                nc.sync.dma_start(out_d[c].transpose([1, 0, 2]), ob[:])


# ======================= host side =======================

_STATE = {}


def _param_tables(Lambda_re, Lambda_im, B, C, D, log_step):
    """f64 host-side parameter/table preparation -> GLOBAL (concat over
    cores along axis 0) input arrays for every tensor except `u`."""
    Lam = Lambda_re.astype(np.float64) + 1j * Lambda_im.astype(np.float64)
    step = np.exp(log_step.astype(np.float64))
    A = np.exp(Lam * step)                        # (P,)
    r = np.abs(A)
    th = np.imag(Lam * step)
    Bt = B[..., 0].astype(np.float64) + 1j * B[..., 1].astype(np.float64)
    Bbar = ((A - 1.0) / Lam)[:, None] * Bt        # (P, H)
    Ct = C[..., 0].astype(np.float64) + 1j * C[..., 1].astype(np.float64)

    s = np.arange(T, dtype=np.float64)
    cs = np.cos(th[:, None] * s[None, :])
    sn = np.sin(th[:, None] * s[None, :])
    rrep = np.broadcast_to(r[:, None], (P, T))
    rpow = r[:, None] ** (s[None, :] + 1.0)

    Br, Bi = np.real(Bbar), np.imag(Bbar)
    btr = np.stack([Br.T[a * P:(a + 1) * P] for a in range(2)])   # (2,128h,128p)
    bti = np.stack([Bi.T[a * P:(a + 1) * P] for a in range(2)])
    crt = 2.0 * np.real(Ct).T                                     # (P, H)
    cit = -2.0 * np.imag(Ct).T
    dd = np.zeros((2, P, H), np.float64)
    for a in range(2):
        for hh in range(P):
            dd[a, hh, a * P + hh] = D[a * P + hh]
    # per-chunk carry application matrices  e^{i th} A^{T c}
    mc = np.zeros((P, 2 * NCH), np.float64)
    eA = np.exp(1j * th)
    for c in range(NCH):
        m = eA * (A ** (T * c))
        mc[:, c] = np.real(m)
        mc[:, NCH + c] = np.imag(m)
    ctc = np.stack([np.cos(th * T), np.sin(th * T)], axis=1)      # (P, 2)

    ALC = A ** LC
    eE = np.exp(1j * th * (T - 1))     # local y -> span-final state phase

    def rep(x):
        # identical per core -> global concat along axis 0
        return np.concatenate([x] * NCORES, axis=0)

    wgr = np.zeros((NCORES, P, NCORES), np.float64)
    wgi = np.zeros((NCORES, P, NCORES), np.float64)
    for i in range(NCORES):
        for j in range(i):
            w = (ALC ** (i - 1 - j)) * eE
            wgr[i, :, j] = np.real(w)
            wgi[i, :, j] = np.imag(w)

    # ---- pack by dtype group (must mirror _body's slice offsets) ----
    cbf = np.empty((P, CBF_W), NPBF16)
    cbf[:, 0:T] = cs.astype(NPBF16)
    cbf[:, T:2 * T] = sn.astype(NPBF16)
    cbf[:, 2 * T:3 * T] = rpow.astype(NPBF16)
    cbf[:, 3 * T:3 * T + H] = crt.astype(NPBF16)
    cbf[:, 3 * T + H:3 * T + 2 * H] = cit.astype(NPBF16)

    cf32 = np.empty((NCORES, P, CF32_W), np.float32)
    cf32[:, :, 0:T] = rrep.astype(np.float32)
    cf32[:, :, T:T + 2 * NCH] = mc.astype(np.float32)
    cf32[:, :, T + 2 * NCH:T + 2 * NCH + 2] = ctc.astype(np.float32)
    o = T + 2 * NCH + 2
    cf32[:, :, o:o + NCORES] = wgr.astype(np.float32)
    cf32[:, :, o + NCORES:o + 2 * NCORES] = wgi.astype(np.float32)

    cfr = np.empty((P, CFR_W), np.float32)
    cfr[:, 0:P] = btr[0].astype(np.float32)
    cfr[:, P:2 * P] = btr[1].astype(np.float32)
    cfr[:, 2 * P:3 * P] = bti[0].astype(np.float32)
    cfr[:, 3 * P:4 * P] = bti[1].astype(np.float32)
    cfr[:, 4 * P:4 * P + H] = dd[0].astype(np.float32)
    cfr[:, 4 * P + H:4 * P + 2 * H] = dd[1].astype(np.float32)
    cfr[:, 4 * P + 2 * H:4 * P + 2 * H + P] = np.eye(P, dtype=np.float32)

    return {
        "cbf": rep(cbf),
        "cf32": cf32.reshape(NCORES * P, CF32_W),
        "cfr": rep(cfr),
    }


def _in_maps_for_sim(inputs):
    """Per-core input maps (for MultiCoreSim in test.py)."""
    tabs = _param_tables(inputs["Lambda_re"], inputs["Lambda_im"],
                         inputs["B"], inputs["C"], inputs["D"],
                         inputs["log_step"])
    u = np.ascontiguousarray(np.asarray(inputs["input_sequence"],
                                        dtype=np.float32))
    u = u.reshape(L // P, P, H)
    ntile = LC // P
    maps = []
    for i in range(NCORES):
        m = {"u": u[i * ntile:(i + 1) * ntile]}
        for name, g in tabs.items():
            per = g.shape[0] // NCORES
            m[name] = g[i * per:(i + 1) * per]
        maps.append(m)
    return maps


def _build_dispatch(nc):
    """Hoisted equivalent of concourse.bass2jax.run_bass_via_pjrt: build
    the shard_map'd jit ONCE so steady-state calls skip retrace/relower."""
    install_neuronx_cc_hook()
    assert nc.dbg_addr is None and not nc.dbg_callbacks
    partition_name = (nc.partition_id_tensor.name
                      if nc.partition_id_tensor else None)

    in_names, out_names, out_avals = [], [], []
    for alloc in nc.m.functions[0].allocations:
        if not isinstance(alloc, mybir.MemoryLocationSet):
            continue
        assert alloc.memorylocations
        name = alloc.memorylocations[0].name
        if alloc.kind == "ExternalInput":
            if name != partition_name:
                in_names.append(name)
        elif alloc.kind == "ExternalOutput":
            assert alloc.tensor_shape is not None and alloc.dtype is not None
            out_names.append(name)
            shape = tuple(alloc.tensor_shape)
            dtype = mybir.dt.np(alloc.dtype)
            out_avals.append(jax.core.ShapedArray(shape, dtype))
    n_params = len(in_names)
    n_outs = len(out_avals)
    all_names = list(in_names) + list(out_names)
    if partition_name is not None:
        all_names.append(partition_name)

    def _kernel_body(*args):
        operands = list(args)
        if partition_name is not None:
            operands.append(partition_id_tensor())
        outs = _bass_exec_p.bind(
            *operands,
            out_avals=tuple(out_avals),
            in_names=tuple(all_names),
            out_names=tuple(out_names),
            lowering_input_output_aliases=(),
            sim_require_finite=True,
            sim_require_nnan=True,
            nc=nc,
        )
        return tuple(outs)

    devices = jax.devices()[:NCORES]
    assert len(devices) == NCORES
    mesh = Mesh(np.asarray(devices), ("core",))
    in_specs = (PartitionSpec("core"),) * (n_params + n_outs)
    out_specs = (PartitionSpec("core"),) * n_outs
    donate = tuple(range(n_params, n_params + n_outs))
    fn = jax.jit(
        shard_map(_kernel_body, mesh=mesh, in_specs=in_specs,
                  out_specs=out_specs, check_rep=False),
        donate_argnums=donate, keep_unused=True,
    )
    return fn, in_names, mesh


def _get_state():
    if "fn" not in _STATE:
        nc = _build_kernel()
        fn, in_names, mesh = _build_dispatch(nc)
        _STATE.update(nc=nc, fn=fn, in_names=in_names, mesh=mesh,
                      sharding=NamedSharding(mesh, PartitionSpec("core")))
    return _STATE


def kernel(Lambda_re, Lambda_im, B, C, D, log_step, input_sequence):
    st = _get_state()
    pkey = (np.asarray(Lambda_re).tobytes(), np.asarray(Lambda_im).tobytes(),
            np.asarray(B).tobytes(), np.asarray(C).tobytes(),
            np.asarray(D).tobytes(), np.asarray(log_step).tobytes())
    if st.get("pkey") != pkey:
        tabs = _param_tables(np.asarray(Lambda_re), np.asarray(Lambda_im),
                             np.asarray(B), np.asarray(C), np.asarray(D),
                             np.asarray(log_step))
        st["const_dev"] = {k: jax.device_put(v, st["sharding"])
                           for k, v in tabs.items()}
        st["pkey"] = pkey
        st["prev_out"] = None

    u = np.ascontiguousarray(np.asarray(input_sequence, dtype=np.float32))
    u = u.reshape(L // P, P, H)        # zero-copy view
    args = [u if name == "u" else st["const_dev"][name]
            for name in st["in_names"]]
    out_buf = st["prev_out"]
    if out_buf is None:
        out_buf = np.zeros((NCORES * NCH, T // P, P, H), np.float32)
    out_arrs = st["fn"](*args, out_buf)
    out_dev = out_arrs[0]
    host = np.asarray(out_dev).reshape(L, H)
    st["prev_out"] = out_dev
    return host


# revision 62
# speedup vs baseline: 1.3435x; 1.0538x over previous
"""Trainium2 Bass kernel for a continuous-time diagonal SSM layer (S5/S4D-style).

Math (see reference):
    A = exp(Lambda * step)                 (P,) complex, |A| = r, arg = theta
    Bu[t] = B_bar @ u[t]                   (L, P) complex
    x[t]  = A * x[t-1] + Bu[t]             diagonal complex scan
    ys    = 2 Re(C_tilde @ x) + D * u

Kernel strategy (8 cores, sequence-parallel over L):
  * core i owns a contiguous span of LC = L/8 timesteps, chunked by T=512.
  * rotation trick: with A = r e^{i.th}, define y[t] = e^{-i.th.t} x[t].
    Then y obeys a REAL per-partition recurrence y[t] = r y[t-1] + w[t],
    w[t] = e^{-i.th.t} Bu[t], which maps onto the DVE tensor_tensor_scan
    instruction (state = data0*state + data1 along the free dim).
  * layout: state-major [128 modes (partitions) x time (free)].  u arrives
    in natural (LC, H) layout and is transposed on-chip via PE-transpose
    (identity matmul) into u^T tiles; Bu^T comes from PE matmuls vs u^T;
    the output matmul produces time-major ys directly (lhsT = x tiles),
    D*u is folded in as a diagonal matmul from u^T.  Phase tables
    (cos/sin of th*s, s in [0,T)) are chunk-local, host-precomputed in
    f64; chunk carries chain through the scan `initial` operand after a
    tiny per-chunk basis rotation.
  * cross-core carry: each core AllGathers its span-final scan state (1KB),
    combines them with host-precomputed decay weights, and adds the
    correction r^{s+1}*G into y before the output projection.
  * dtypes: bf16 elementwise domain, f32 scan multiplier (bf16 cannot
    represent slow decay rates like r=0.99995), float32r matmuls.

Host strategy (the per-call wall-clock is dominated by host work, not the
~100us device program):
  * the jax/PJRT dispatch (shard_map over 8 cores -> bass_exec custom
    call) is built ONCE and cached; steady-state calls hit the jit C++
    fast path instead of re-tracing + re-lowering per call.
  * all parameter-derived tables are cached keyed by the raw param bytes
    and kept device-resident (device_put once); only `u` (the activation)
    crosses host->device per call, with no host-side reshape/transpose
    (the global input IS input_sequence).
  * the previous call's device-resident output is donated as the next
    call's output buffer, so no zero-buffer upload per call.
"""

import numpy as np
import ml_dtypes
import jax
from jax.experimental.shard_map import shard_map
from jax.sharding import Mesh, NamedSharding, PartitionSpec

import concourse.bass as bass
import concourse.bacc as bacc
import concourse.tile as tile
import concourse.mybir as mybir
from concourse.bass2jax import (
    _bass_exec_p,
    install_neuronx_cc_hook,
    partition_id_tensor,
)

F32 = mybir.dt.float32
F32R = mybir.dt.float32r
BF16 = mybir.dt.bfloat16
NPBF16 = ml_dtypes.bfloat16
AX = mybir.AxisListType.X
MUL = mybir.AluOpType.mult
ADD = mybir.AluOpType.add

L, H, P = 32768, 256, 128
NCORES = 8
LC = L // NCORES          # 4096 timesteps per core
T = 512                   # chunk length
NCH = LC // T             # 8 chunks per core
KT = T // P               # 4 row-tiles of u per chunk

# packed-constant layouts (free-dim offsets)
CBF_W = 3 * T + 2 * H     # cs | sn | rpow | crt | cit
CF32_W = T + 3 + 2 * NCH + 2 + 2 * NCORES  # rrep | r | D(2) | mc | ctc | wg*
CFR_W = 5 * P + 2 * H                   # ident | btr(2) | bti(2) | dd(2)


def _build_kernel(single=False):
    nc = bacc.Bacc(
        "TRN2", target_bir_lowering=False, debug=False,
        enable_asserts=False, num_devices=1 if single else NCORES,
    )
    # ---------------- per-core external I/O ----------------
    # u in natural layout, viewed as [row-tiles, P, H] for permuted DMA
    u_d = nc.dram_tensor("u", [LC // P, P, H], F32R, kind="ExternalInput").ap()
    # constants packed by dtype into single tensors (one DMA each)
    cbf_d = nc.dram_tensor("cbf", [P, CBF_W], BF16, kind="ExternalInput").ap()
    cf32_d = nc.dram_tensor("cf32", [P, CF32_W], F32, kind="ExternalInput").ap()
    cfr_d = nc.dram_tensor("cfr", [P, CFR_W], F32R, kind="ExternalInput").ap()
    out_d = nc.dram_tensor("out", [NCH, T // P, P, H], F32,
                           kind="ExternalOutput").ap()

    with tile.TileContext(nc) as tc:
        _body(tc, nc, u_d, cbf_d, cf32_d, cfr_d, out_d, single=single)
    nc.compile()
    return nc


def _body(tc, nc, u_d, cbf_d, cf32_d, cfr_d, out_d, single=False):
    with (
        tc.tile_pool(name="const", bufs=1) as cpool,
        tc.tile_pool(name="span", bufs=1) as spool,
        tc.tile_pool(name="work", bufs=5) as wpool,
        tc.tile_pool(name="dram", bufs=1, space="DRAM") as dpool,
    ):
        # ---- packed constants: one DMA per dtype group ----
        cbf = cpool.tile([P, CBF_W], BF16)
        cf32 = cpool.tile([P, CF32_W], F32)
        cfr = cpool.tile([P, CFR_W], F32R)
        u0 = cpool.tile([P, KT, H], F32R)      # chunk-0 u, ahead of consts
        nc.sync.dma_start(u0[:], u_d[0:KT].transpose([1, 0, 2]))
        # ident/btr/bti first (gates the very first PE work); dd later
        nc.sync.dma_start(cfr[:, 0:5 * P], cfr_d[:, 0:5 * P])
        nc.sync.dma_start(cbf[:], cbf_d)
        nc.sync.dma_start(cfr[:, 5 * P:], cfr_d[:, 5 * P:])
        nc.sync.dma_start(cf32[:], cf32_d)
        cs_sb = cbf[:, 0:T]
        sn_sb = cbf[:, T:2 * T]
        rpow_sb = cbf[:, 2 * T:3 * T]
        crt_sb = cbf[:, 3 * T:3 * T + H]
        cit_sb = cbf[:, 3 * T + H:3 * T + 2 * H]
        rrep_sb = cf32[:, 0:T]
        dcol = cf32[:, T + 1:T + 3]
        o = T + 3
        mc_sb = cf32[:, o:o + 2 * NCH]
        ctc_sb = cf32[:, o + 2 * NCH:o + 2 * NCH + 2]
        wgr_sb = cf32[:, o + 2 * NCH + 2:o + 2 * NCH + 2 + NCORES]
        wgi_sb = cf32[:, o + 2 * NCH + 2 + NCORES:
                      o + 2 * NCH + 2 + 2 * NCORES]
        ident = cfr[:, 0:P]
        btr_sb = [cfr[:, P + a * P:P + (a + 1) * P] for a in range(2)]
        bti_sb = [cfr[:, 3 * P + a * P:3 * P + (a + 1) * P] for a in range(2)]
        dd_sb = [cfr[:, 5 * P + a * H:5 * P + (a + 1) * H] for a in range(2)]

        # ---- span-persistent state ----
        ut_sb = spool.tile([P, 2, LC], F32R)      # u^T, built on-chip
        yr_sb = spool.tile([P, LC], BF16)         # scan outputs (rotated basis)
        yi_sb = spool.tile([P, LC], BF16)
        gcols = spool.tile([P, 2 * (NCH + 1)], F32)   # chunk-carry columns


        # =============== phase 1: transpose u, Bu, rotate, scan ===============
        with (
            tc.tile_pool(name="psum", bufs=2, space="PSUM") as ppool,
            tc.tile_pool(name="tpsum", bufs=2, space="PSUM") as tpool,
        ):
            for c in range(NCH):
                t0 = c * T
                # one permuted DMA brings u[t0:t0+T, :] as [P, KT, H]
                if c == 0:
                    un4 = u0
                else:
                    un4 = wpool.tile([P, KT, H], F32R, tag="un")
                    nc.sync.dma_start(
                        un4[:], u_d[c * KT:(c + 1) * KT].transpose([1, 0, 2]))
                # PE-transpose into one PSUM bank per half, single copy out
                # (copy engines split: Act for half 0, DVE for half 1)
                for hh in range(2):
                    pt4 = tpool.tile([P, T], F32R, tag=f"pt{hh}")
                    for k in range(KT):
                        nc.tensor.transpose(
                            pt4[:, k * P:(k + 1) * P],
                            un4[:, k, hh * P:(hh + 1) * P], ident)
                    nc.scalar.copy(ut_sb[:, hh, t0:t0 + T], pt4[:])
                pbr = ppool.tile([P, T], F32, tag="pbur")
                pbi = ppool.tile([P, T], F32, tag="pbui")
                for a in range(2):
                    nc.tensor.matmul(pbr[:], btr_sb[a],
                                     ut_sb[:, a, t0:t0 + T],
                                     start=(a == 0), stop=(a == 1))
                for a in range(2):
                    nc.tensor.matmul(pbi[:], bti_sb[a],
                                     ut_sb[:, a, t0:t0 + T],
                                     start=(a == 0), stop=(a == 1))
                # w = e^{-i th s} * Bu
                # (Pool/gpsimd cannot read PSUM on HW: stage via Act copies)
                bur = wpool.tile([P, T], BF16, tag="bur")
                bui = wpool.tile([P, T], BF16, tag="bui")
                nc.scalar.copy(bur[:], pbr[:])
                nc.scalar.copy(bui[:], pbi[:])
                m1 = wpool.tile([P, T], BF16, tag="m1")
                m2 = wpool.tile([P, T], BF16, tag="m2")
                m3 = wpool.tile([P, T], BF16, tag="m3")
                m4 = wpool.tile([P, T], BF16, tag="m4")
                wr = wpool.tile([P, T], BF16, tag="wr")
                wi = wpool.tile([P, T], BF16, tag="wi")
                nc.vector.tensor_mul(m1[:], cs_sb, bur[:])
                nc.gpsimd.tensor_mul(m2[:], sn_sb, bui[:])
                nc.vector.tensor_add(wr[:], m1[:], m2[:])
                nc.vector.tensor_mul(m3[:], cs_sb, bui[:])
                nc.gpsimd.tensor_mul(m4[:], sn_sb, bur[:])
                nc.vector.tensor_sub(wi[:], m3[:], m4[:])
                # scans (DVE only: Pool scans are ~3.4x slower in the
                # cost model and would serialize the chunk chain)
                init_r = 0.0 if c == 0 else gcols[:, 2 * c:2 * c + 1]
                init_i = 0.0 if c == 0 else gcols[:, 2 * c + 1:2 * c + 2]
                nc.vector.tensor_tensor_scan(
                    yr_sb[:, t0:t0 + T], rrep_sb, wr[:], init_r,
                    op0=MUL, op1=ADD)
                nc.vector.tensor_tensor_scan(
                    yi_sb[:, t0:t0 + T], rrep_sb, wi[:], init_i,
                    op0=MUL, op1=ADD)
                # chunk carry: g_{c+1} = e^{i th T} * y[:, last]
                if c < NCH - 1:
                    ylr = yr_sb[:, t0 + T - 1:t0 + T]
                    yli = yi_sb[:, t0 + T - 1:t0 + T]
                    tc1 = wpool.tile([P, 1], F32, tag="tc1")
                    tc2 = wpool.tile([P, 1], F32, tag="tc2")
                    nc.vector.tensor_scalar_mul(tc1[:], yli[:], ctc_sb[:, 1:2])
                    nc.vector.scalar_tensor_tensor(
                        gcols[:, 2 * c + 2:2 * c + 3], ylr, ctc_sb[:, 0:1],
                        tc1[:], op0=MUL, op1=mybir.AluOpType.subtract)
                    nc.vector.tensor_scalar_mul(tc2[:], ylr[:], ctc_sb[:, 1:2])
                    nc.vector.scalar_tensor_tensor(
                        gcols[:, 2 * c + 3:2 * c + 4], yli, ctc_sb[:, 0:1],
                        tc2[:], op0=MUL, op1=ADD)

        # =============== cross-core carry exchange ===============
        e_loc = dpool.tile([P, 2], F32)
        e_all = dpool.tile([NCORES, P, 2], F32)
        e_sb = cpool.tile([P, 2], F32)
        nc.vector.tensor_copy(e_sb[:, 0:1], yr_sb[:, LC - 1:LC])
        nc.vector.tensor_copy(e_sb[:, 1:2], yi_sb[:, LC - 1:LC])
        nc.sync.dma_start(e_loc[:], e_sb[:])
        if single:
            nc.gpsimd.dma_start(e_all[0], e_loc[:])
        else:
            nc.gpsimd.collective_compute(
                "AllGather", mybir.AluOpType.bypass,
                replica_groups=[list(range(NCORES))],
                ins=[e_loc.opt()], outs=[e_all.opt()])
        # one permuted DMA gathers all spans' finals as [P, NCORES, 2]
        eall_sb = cpool.tile([P, NCORES, 2], F32)
        nc.sync.dma_start(eall_sb[:], e_all.transpose([1, 0, 2]))
        # Cin = sum_j W'_j * E_j   (complex, W' host-precomputed per core)
        # real path on DVE, imag path on Pool — independent chains
        er_v = eall_sb[:, :, 0]
        ei_v = eall_sb[:, :, 1]
        ta = cpool.tile([P, NCORES], F32)
        tb = cpool.tile([P, NCORES], F32)
        ta2 = cpool.tile([P, NCORES], F32)
        tb2 = cpool.tile([P, NCORES], F32)
        cin = cpool.tile([P, 2], F32)
        nc.vector.tensor_mul(ta[:], wgr_sb, er_v)
        nc.vector.tensor_mul(tb[:], wgi_sb, ei_v)
        nc.vector.tensor_sub(ta[:], ta[:], tb[:])
        nc.vector.tensor_reduce(cin[:, 0:1], ta[:], axis=AX, op=ADD)
        nc.gpsimd.tensor_mul(ta2[:], wgi_sb, er_v)
        nc.gpsimd.tensor_mul(tb2[:], wgr_sb, ei_v)
        nc.gpsimd.tensor_add(ta2[:], ta2[:], tb2[:])
        nc.vector.tensor_reduce(cin[:, 1:2], ta2[:], axis=AX, op=ADD)
        # G_all[:, c] = (e^{i th} A^{T c}) * Cin  for each chunk c
        gr_all = cpool.tile([P, NCH], F32)
        gi_all = cpool.tile([P, NCH], F32)
        tg = cpool.tile([P, NCH], F32)
        tg2 = cpool.tile([P, NCH], F32)
        nc.vector.tensor_scalar_mul(gr_all[:], mc_sb[:, 0:NCH], cin[:, 0:1])
        nc.vector.tensor_scalar_mul(tg[:], mc_sb[:, NCH:2 * NCH], cin[:, 1:2])
        nc.vector.tensor_sub(gr_all[:], gr_all[:], tg[:])
        nc.gpsimd.tensor_scalar_mul(gi_all[:], mc_sb[:, NCH:2 * NCH],
                                    cin[:, 0:1])
        nc.gpsimd.tensor_scalar_mul(tg2[:], mc_sb[:, 0:NCH], cin[:, 1:2])
        nc.gpsimd.tensor_add(gi_all[:], gi_all[:], tg2[:])


        # =============== phase 2: correct, unrotate, project ===============
        with tc.tile_pool(name="opsum", bufs=2, space="PSUM") as opool:
            for c in range(NCH):
                t0 = c * T
                # y += r^{s+1} * G_c
                cr1 = wpool.tile([P, T], BF16, tag="cr1")
                cr2 = wpool.tile([P, T], BF16, tag="cr2")
                yrc = wpool.tile([P, T], BF16, tag="yrc")
                yic = wpool.tile([P, T], BF16, tag="yic")
                nc.scalar.mul(cr1[:], rpow_sb, gr_all[:, c:c + 1])
                nc.scalar.mul(cr2[:], rpow_sb, gi_all[:, c:c + 1])
                nc.vector.tensor_add(yrc[:], cr1[:], yr_sb[:, t0:t0 + T])
                nc.vector.tensor_add(yic[:], cr2[:], yi_sb[:, t0:t0 + T])
                # x = e^{+i th s} * y
                n1 = wpool.tile([P, T], BF16, tag="n1")
                n2 = wpool.tile([P, T], BF16, tag="n2")
                xr = wpool.tile([P, T], BF16, tag="xr")
                xi = wpool.tile([P, T], BF16, tag="xi")
                nc.vector.tensor_mul(n1[:], cs_sb, yrc[:])
                nc.gpsimd.tensor_mul(n2[:], sn_sb, yic[:])
                nc.vector.tensor_sub(xr[:], n1[:], n2[:])
                nc.gpsimd.tensor_mul(n1[:], cs_sb, yic[:])
                nc.vector.tensor_mul(n2[:], sn_sb, yrc[:])
                nc.vector.tensor_add(xi[:], n1[:], n2[:])
                # ys^T-free output: ys[t,h] = 2Re(C x)[t,h] + (D u)[t,h]
                po = opool.tile([P, 4, H], F32, tag="po")
                ob = wpool.tile([P, 4, H], F32, tag="ob")
                for a in range(4):
                    sl = slice(a * P, (a + 1) * P)
                    nc.tensor.matmul(po[:, a, :], xr[:, sl], crt_sb,
                                     start=True, stop=False)
                    nc.tensor.matmul(po[:, a, :], xi[:, sl], cit_sb,
                                     start=False, stop=False)
                    nc.tensor.matmul(po[:, a, :],
                                     ut_sb[:, 0, t0 + a * P:t0 + (a + 1) * P],
                                     dd_sb[0], start=False, stop=False)
                    nc.tensor.matmul(po[:, a, :],
                                     ut_sb[:, 1, t0 + a * P:t0 + (a + 1) * P],
                                     dd_sb[1], start=False, stop=True)
                nc.scalar.copy(ob[:], po[:])
                nc.sync.dma_start(out_d[c].transpose([1, 0, 2]), ob[:])


# ======================= host side =======================

_STATE = {}


def _param_tables(Lambda_re, Lambda_im, B, C, D, log_step):
    """f64 host-side parameter/table preparation -> GLOBAL (concat over
    cores along axis 0) input arrays for every tensor except `u`."""
    Lam = Lambda_re.astype(np.float64) + 1j * Lambda_im.astype(np.float64)
    step = np.exp(log_step.astype(np.float64))
    A = np.exp(Lam * step)                        # (P,)
    r = np.abs(A)
    th = np.imag(Lam * step)
    Bt = B[..., 0].astype(np.float64) + 1j * B[..., 1].astype(np.float64)
    Bbar = ((A - 1.0) / Lam)[:, None] * Bt        # (P, H)
    Ct = C[..., 0].astype(np.float64) + 1j * C[..., 1].astype(np.float64)

    s = np.arange(T, dtype=np.float64)
    cs = np.cos(th[:, None] * s[None, :])
    sn = np.sin(th[:, None] * s[None, :])
    rpow = r[:, None] ** (s[None, :] + 1.0)

    Br, Bi = np.real(Bbar), np.imag(Bbar)
    btr = np.stack([Br.T[a * P:(a + 1) * P] for a in range(2)])   # (2,128h,128p)
    bti = np.stack([Bi.T[a * P:(a + 1) * P] for a in range(2)])
    crt = 2.0 * np.real(Ct).T                                     # (P, H)
    cit = -2.0 * np.imag(Ct).T
    # per-chunk carry application matrices  e^{i th} A^{T c}
    mc = np.zeros((P, 2 * NCH), np.float64)
    eA = np.exp(1j * th)
    for c in range(NCH):
        m = eA * (A ** (T * c))
        mc[:, c] = np.real(m)
        mc[:, NCH + c] = np.imag(m)
    ctc = np.stack([np.cos(th * T), np.sin(th * T)], axis=1)      # (P, 2)

    ALC = A ** LC
    eE = np.exp(1j * th * (T - 1))     # local y -> span-final state phase

    def rep(x):
        # identical per core -> global concat along axis 0
        return np.concatenate([x] * NCORES, axis=0)

    wgr = np.zeros((NCORES, P, NCORES), np.float64)
    wgi = np.zeros((NCORES, P, NCORES), np.float64)
    for i in range(NCORES):
        for j in range(i):
            w = (ALC ** (i - 1 - j)) * eE
            wgr[i, :, j] = np.real(w)
            wgi[i, :, j] = np.imag(w)

    # ---- pack by dtype group (must mirror _body's slice offsets) ----
    cbf = np.empty((P, CBF_W), NPBF16)
    cbf[:, 0:T] = cs.astype(NPBF16)
    cbf[:, T:2 * T] = sn.astype(NPBF16)
    cbf[:, 2 * T:3 * T] = rpow.astype(NPBF16)
    cbf[:, 3 * T:3 * T + H] = crt.astype(NPBF16)
    cbf[:, 3 * T + H:3 * T + 2 * H] = cit.astype(NPBF16)

    cf32 = np.empty((NCORES, P, CF32_W), np.float32)
    cf32[:, :, 0:T] = r[:, None].astype(np.float32)
    cf32[:, :, T] = r.astype(np.float32)
    cf32[:, :, T + 1] = D[:P].astype(np.float32)
    cf32[:, :, T + 2] = D[P:].astype(np.float32)
    o = T + 3
    cf32[:, :, o:o + 2 * NCH] = mc.astype(np.float32)
    cf32[:, :, o + 2 * NCH:o + 2 * NCH + 2] = ctc.astype(np.float32)
    o = o + 2 * NCH + 2
    cf32[:, :, o:o + NCORES] = wgr.astype(np.float32)
    cf32[:, :, o + NCORES:o + 2 * NCORES] = wgi.astype(np.float32)

    dd = np.zeros((2, P, H), np.float64)
    for a in range(2):
        dd[a, np.arange(P), a * P + np.arange(P)] = D[a * P:(a + 1) * P]
    cfr = np.empty((P, CFR_W), np.float32)
    cfr[:, 0:P] = np.eye(P, dtype=np.float32)
    cfr[:, P:2 * P] = btr[0].astype(np.float32)
    cfr[:, 2 * P:3 * P] = btr[1].astype(np.float32)
    cfr[:, 3 * P:4 * P] = bti[0].astype(np.float32)
    cfr[:, 4 * P:5 * P] = bti[1].astype(np.float32)
    cfr[:, 5 * P:5 * P + H] = dd[0].astype(np.float32)
    cfr[:, 5 * P + H:5 * P + 2 * H] = dd[1].astype(np.float32)

    return {
        "cbf": rep(cbf),
        "cf32": cf32.reshape(NCORES * P, CF32_W),
        "cfr": rep(cfr),
    }


def _in_maps_for_sim(inputs):
    """Per-core input maps (for MultiCoreSim in test.py)."""
    tabs = _param_tables(inputs["Lambda_re"], inputs["Lambda_im"],
                         inputs["B"], inputs["C"], inputs["D"],
                         inputs["log_step"])
    u = np.ascontiguousarray(np.asarray(inputs["input_sequence"],
                                        dtype=np.float32))
    u = u.reshape(L // P, P, H)
    ntile = LC // P
    maps = []
    for i in range(NCORES):
        m = {"u": u[i * ntile:(i + 1) * ntile]}
        for name, g in tabs.items():
            per = g.shape[0] // NCORES
            m[name] = g[i * per:(i + 1) * per]
        maps.append(m)
    return maps


def _build_dispatch(nc):
    """Hoisted equivalent of concourse.bass2jax.run_bass_via_pjrt: build
    the shard_map'd jit ONCE so steady-state calls skip retrace/relower."""
    install_neuronx_cc_hook()
    assert nc.dbg_addr is None and not nc.dbg_callbacks
    partition_name = (nc.partition_id_tensor.name
                      if nc.partition_id_tensor else None)

    in_names, out_names, out_avals = [], [], []
    for alloc in nc.m.functions[0].allocations:
        if not isinstance(alloc, mybir.MemoryLocationSet):
            continue
        assert alloc.memorylocations
        name = alloc.memorylocations[0].name
        if alloc.kind == "ExternalInput":
            if name != partition_name:
                in_names.append(name)
        elif alloc.kind == "ExternalOutput":
            assert alloc.tensor_shape is not None and alloc.dtype is not None
            out_names.append(name)
            shape = tuple(alloc.tensor_shape)
            dtype = mybir.dt.np(alloc.dtype)
            out_avals.append(jax.core.ShapedArray(shape, dtype))
    n_params = len(in_names)
    n_outs = len(out_avals)
    all_names = list(in_names) + list(out_names)
    if partition_name is not None:
        all_names.append(partition_name)

    def _kernel_body(*args):
        operands = list(args)
        if partition_name is not None:
            operands.append(partition_id_tensor())
        outs = _bass_exec_p.bind(
            *operands,
            out_avals=tuple(out_avals),
            in_names=tuple(all_names),
            out_names=tuple(out_names),
            lowering_input_output_aliases=(),
            sim_require_finite=True,
            sim_require_nnan=True,
            nc=nc,
        )
        return tuple(outs)

    devices = jax.devices()[:NCORES]
    assert len(devices) == NCORES
    mesh = Mesh(np.asarray(devices), ("core",))
    in_specs = (PartitionSpec("core"),) * (n_params + n_outs)
    out_specs = (PartitionSpec("core"),) * n_outs
    donate = tuple(range(n_params, n_params + n_outs))
    fn = jax.jit(
        shard_map(_kernel_body, mesh=mesh, in_specs=in_specs,
                  out_specs=out_specs, check_rep=False),
        donate_argnums=donate, keep_unused=True,
    )
    return fn, in_names, mesh


def _get_state():
    if "fn" not in _STATE:
        nc = _build_kernel()
        fn, in_names, mesh = _build_dispatch(nc)
        _STATE.update(nc=nc, fn=fn, in_names=in_names, mesh=mesh,
                      sharding=NamedSharding(mesh, PartitionSpec("core")))
    return _STATE


def kernel(Lambda_re, Lambda_im, B, C, D, log_step, input_sequence):
    st = _get_state()
    pkey = (np.asarray(Lambda_re).tobytes(), np.asarray(Lambda_im).tobytes(),
            np.asarray(B).tobytes(), np.asarray(C).tobytes(),
            np.asarray(D).tobytes(), np.asarray(log_step).tobytes())
    if st.get("pkey") != pkey:
        tabs = _param_tables(np.asarray(Lambda_re), np.asarray(Lambda_im),
                             np.asarray(B), np.asarray(C), np.asarray(D),
                             np.asarray(log_step))
        st["const_dev"] = {k: jax.device_put(v, st["sharding"])
                           for k, v in tabs.items()}
        st["pkey"] = pkey
        st["prev_out"] = None

    u = np.ascontiguousarray(np.asarray(input_sequence, dtype=np.float32))
    u = u.reshape(L // P, P, H)        # zero-copy view
    args = [u if name == "u" else st["const_dev"][name]
            for name in st["in_names"]]
    out_buf = st["prev_out"]
    if out_buf is None:
        out_buf = np.zeros((NCORES * NCH, T // P, P, H), np.float32)
    out_arrs = st["fn"](*args, out_buf)
    out_dev = out_arrs[0]
    host = np.asarray(out_dev).reshape(L, H)
    st["prev_out"] = out_dev
    return host
